# revision 8
# baseline (speedup 1.0000x reference)
"""Balanced CE loss + accuracy on 8 Trainium2 NeuronCores (Bass/Tile).

Reference computation (N = 16777216 elements):
    loss = -sum(where(t==1, 1.6*log(p), 0.4*log(1-p))) / N
    acc  = mean(round(p) == t)

Strategy (data-parallel over N, no collectives needed):
  Shard N across 8 cores. Per core, stream [128, 4096] tiles and compute,
  using the identity log(1)=0 to avoid an elementwise select:
    y1 = p if t==1 else 1      -> sum(ln(y1)) = sum_{t==1} ln(p)     =: A1
    y0 = 1-p if t==0 else 1    -> sum(ln(y0)) = sum_{t==0} ln(1-p)   =: B0
  with a1 = (p-1)*t = y1-1 and a0 = (t-1)*p = y0-1, each one fused DVE
  scalar_tensor_tensor op; ACT computes Ln(a+1) with fused free-dim
  accumulation. Accuracy from two threshold counts:
    C1 = #(y1 >= 0.5) (DVE tensor_scalar is_ge + accum)
    S0 = sum(sign(y0 - 0.5)) (ACT Sign + accum), C0 = (S0 + N)/2
    correct = C1 + C0 - N
  Per-(partition, tile) partials are DMA'd out; host reduces in float64:
    loss = -(1.6*A1 + 0.4*B0)/N, acc = correct/N.
"""

import sys

if "/opt/trn_rl_repo" not in sys.path:
    sys.path.insert(0, "/opt/trn_rl_repo")

import numpy as np

import concourse.bass as bass
import concourse.bacc as bacc
import concourse.tile as tile
from concourse import mybir
from concourse.bass_utils import run_bass_kernel_spmd

N_CORES = 8
N = 16777216
P = 128
F = 4096
NT = N // (N_CORES * P * F)  # tiles per core = 4

AF = mybir.ActivationFunctionType
OP = mybir.AluOpType

_NC_CACHE = None


def build_bass():
    """Build the single-core Bass program (SPMD across 8 cores)."""
    global _NC_CACHE
    if _NC_CACHE is not None:
        return _NC_CACHE

    nc = bacc.Bacc("TRN2", target_bir_lowering=False, debug=False)

    # const AP for the Sign bias ln(2) (only 0.0/1.0 are pre-registered)
    LN2 = 0.6931471805599453
    _c = nc.alloc_sbuf_tensor("const-float32-ln2", [128, 1], mybir.dt.float32)
    nc.gpsimd.memset(_c.ap(), LN2)
    nc.const_aps.aps[(mybir.dt.float32, LN2)] = _c.ap()
    nc.all_engine_barrier()

    p_in = nc.dram_tensor("p_in", [NT, P, F], mybir.dt.float32, kind="ExternalInput").ap()
    t_in = nc.dram_tensor("t_in", [NT, P, F], mybir.dt.int32, kind="ExternalInput").ap()
    # acc_act columns: [0:NT) = sum ln(y1), [NT:2NT) = sum ln(y0), [2NT:3NT) = sum sign(y0-0.5)
    acc_act = nc.dram_tensor("acc_act", [P, 3 * NT], mybir.dt.float32, kind="ExternalOutput").ap()
    # acc_dve columns: [0:NT) = count(y1 >= 0.5)
    acc_dve = nc.dram_tensor("acc_dve", [P, NT], mybir.dt.float32, kind="ExternalOutput").ap()

    with tile.TileContext(nc) as tc:
        with (
            tc.tile_pool(name="io", bufs=2) as io_pool,
            tc.tile_pool(name="work", bufs=2) as work_pool,
            tc.tile_pool(name="dump", bufs=1) as dump_pool,
            tc.tile_pool(name="acc", bufs=1) as acc_pool,
        ):
            dvedump = dump_pool.tile([P, F], mybir.dt.float32, tag="dvedump")
            acc_act_sb = acc_pool.tile([P, 3 * NT], mybir.dt.float32, tag="acca")
            acc_dve_sb = acc_pool.tile([P, NT], mybir.dt.float32, tag="accd")

            for i in range(NT):
                p_t = io_pool.tile([P, F], mybir.dt.float32, tag="p")
                t_t = io_pool.tile([P, F], mybir.dt.int32, tag="t")
                nc.sync.dma_start(p_t[:], p_in[i])
                nc.sync.dma_start(t_t[:], t_in[i])

                a1 = work_pool.tile([P, F], mybir.dt.float32, tag="a1")
                a0 = work_pool.tile([P, F], mybir.dt.float32, tag="a0")
                # a1 = (p - 1) * t ;  a0 = (t - 1) * p   (t int32, converted on read)
                nc.vector.scalar_tensor_tensor(a1[:], p_t[:], -1.0, t_t[:], OP.add, OP.mult)
                nc.vector.scalar_tensor_tensor(a0[:], t_t[:], -1.0, p_t[:], OP.add, OP.mult)
                # C1 partial: count(a1 >= -0.5)  (runs 2x on DVE)
                nc.vector.tensor_scalar(dvedump[:], a1[:], -0.5, None, OP.is_ge,
                                        OP.add, accum_out=acc_dve_sb[:, i : i + 1])
                # ACT ops write in-place so each needs at most one sem wait.
                # A1 partial: a1 <- ln(a1 + 1); accum = sum ln(y1)
                nc.scalar.activation(a1[:], a1[:], AF.Ln, bias=1.0,
                                     accum_out=acc_act_sb[:, i : i + 1])
                # B0 partial: a0 <- ln(a0 + 1); accum = sum ln(y0)
                nc.scalar.activation(a0[:], a0[:], AF.Ln, bias=1.0,
                                     accum_out=acc_act_sb[:, NT + i : NT + i + 1])
                # S0 partial: sum sign(ln(y0) + ln 2)  == sum sign(y0 - 0.5)
                nc.scalar.activation(a0[:], a0[:], AF.Sign, bias=LN2,
                                     accum_out=acc_act_sb[:, 2 * NT + i : 2 * NT + i + 1])

            nc.sync.dma_start(acc_act[:], acc_act_sb[:])
            nc.sync.dma_start(acc_dve[:], acc_dve_sb[:])

    nc.finalize()
    _NC_CACHE = nc
    return nc


def make_in_maps(input, target):
    inp = np.ascontiguousarray(np.asarray(input, dtype=np.float32)).reshape(
        N_CORES, NT, P, F
    )
    tgt = np.ascontiguousarray(np.asarray(target, dtype=np.int32)).reshape(
        N_CORES, NT, P, F
    )
    return [{"p_in": inp[c], "t_in": tgt[c]} for c in range(N_CORES)]


def combine(results):
    """Host-side unshard: reduce the 8 cores' partial sums -> (loss, acc)."""
    A1 = B0 = S0 = C1 = 0.0
    for r in results:
        aa = np.asarray(r["acc_act"], dtype=np.float64)
        ad = np.asarray(r["acc_dve"], dtype=np.float64)
        A1 += aa[:, 0:NT].sum()
        B0 += aa[:, NT : 2 * NT].sum()
        S0 += aa[:, 2 * NT : 3 * NT].sum()
        C1 += ad.sum()
    loss = -(1.6 * A1 + 0.4 * B0) / N
    C0 = (S0 + N) / 2.0
    acc = (C1 + C0 - N) / N
    return np.float32(loss), np.float32(acc)


def run_on_hw(input, target, **spmd_kwargs):
    nc = build_bass()
    in_maps = make_in_maps(input, target)
    return run_bass_kernel_spmd(nc, in_maps, list(range(N_CORES)), **spmd_kwargs)


def kernel(input, target):
    br = run_on_hw(input, target)
    return combine(br.results)
